# revision 4
# baseline (speedup 1.0000x reference)
"""Trainium2 Bass kernel for nn_DialogRater (RGCN message passing), v3.

Contract: kernel(**inputs) takes the FULL unsharded inputs and returns the
FULL output [256, 4] float32.

Strategy (8 NeuronCores, SPMD, all-bf16 matmul datapath):
  - Nodes partitioned by graph: 32 graphs of 256 nodes per core, assigned
    to (core, slot) cells balancing fat relations across cores.
  - Host materializes, per core: gathered edge features xg = x[src]/cnt
    (bf16, the halo/feature exchange at sharding time), one-hot scatter
    tiles (exact 0/1), transposed root-feature tiles, stacked weights.
  - Device, per set of 2 graphs (512 node columns):
      agg:  Ut[r][k] += G_tile[:,k].T @ onehot_tile  (PE; 128-edge tiles,
            256-wide one-hot per (graph, relation)), evicted f32->bf16
            into Ball[128, 30 slots, 512] (DVE/ACT round-robin); root x.T
            tiles DMA'd into slots 27..29.
      xform: for each 128-feature chunk Hc: hT = sum_slot W[slot,Hc].T @
            Ball[slot]  (PE, 30-slot psum accumulation, N=512, output
            transposed: features x nodes), relu on ACT (+b_conv bias),
            per-graph pool via free-axis segmented reduce on DVE.
  - Tiny epilogue (mean /256, lin1, BatchNorm over 256 graphs, head) on
    host in float64.
"""
import sys

sys.path.insert(0, "/opt/trn_rl_repo")

from contextlib import ExitStack

import numpy as np
import ml_dtypes

import concourse.bass as bass
import concourse.tile as tile
from concourse import bacc, mybir
from concourse.bass_utils import run_bass_kernel_spmd

NC = 8
N_NODES = 65536
D = 384
H = 384
N_REL = 9
GRAPH = 256                     # nodes per graph = one-hot width
NSLOT = 32                      # graphs per core
SETS = 16                       # 2 graphs per set
P = 128
TILE_E = 128                    # edges per tile
GB = 6                          # edge tiles per DMA batch
BN_EPS = 1e-5

bf16 = ml_dtypes.bfloat16

LAST_RES = None

OH_DTYPE_NP = bf16
OH_DTYPE_BIR = mybir.dt.bfloat16


def _pack_graphs(cnt_gr):
    """Assign graphs to (core, slot) cells, packing graphs that are fat in
    the same relation into the same slot to minimize cross-core tile max."""
    n_graphs = N_NODES // GRAPH
    fat = np.ceil(cnt_gr / TILE_E).astype(np.int64) > 2
    unassigned = set(range(n_graphs))
    slots = []
    for r in np.argsort(-fat.sum(axis=0)):
        fg = [g for g in range(n_graphs) if g in unassigned and fat[g, r]]
        while len(fg) >= NC:
            s, fg = fg[:NC], fg[NC:]
            slots.append(s)
            unassigned -= set(s)
    rest = sorted(unassigned,
                  key=lambda g: (-fat[g].sum(), tuple(np.where(fat[g])[0])))
    while rest:
        slots.append(rest[:NC])
        rest = rest[NC:]
    graph_core = np.zeros(n_graphs, np.int64)
    graph_slot = np.zeros(n_graphs, np.int64)
    graph_of = [[0] * NSLOT for _ in range(NC)]
    for j, slot in enumerate(slots):
        for c, g in enumerate(slot):
            graph_core[g] = c
            graph_slot[g] = j
            graph_of[c][j] = g
    return graph_core, graph_slot, graph_of


def _preprocess(src, dst, et):
    cnt = np.bincount(dst * N_REL + et, minlength=N_NODES * N_REL).reshape(
        N_NODES, N_REL)
    invc = (1.0 / np.maximum(cnt, 1.0)).astype(np.float32)

    n_graphs = N_NODES // GRAPH
    g_of_edge = dst // GRAPH
    cnt_gr = np.bincount(g_of_edge * N_REL + et,
                         minlength=n_graphs * N_REL).reshape(n_graphs, N_REL)
    graph_core, graph_slot, graph_of = _pack_graphs(cnt_gr)

    core = graph_core[g_of_edge]
    slot = graph_slot[g_of_edge]
    col = dst % GRAPH
    st = slot // 2
    pos = slot % 2
    gid = ((core * SETS + st) * 2 + pos) * N_REL + et
    n_groups = NC * SETS * 2 * N_REL
    order = np.argsort(gid, kind="stable")
    gid_s = gid[order]
    counts = np.bincount(gid_s, minlength=n_groups).reshape(
        NC, SETS * 2 * N_REL)
    ntiles = np.maximum(
        np.ceil(counts / TILE_E).astype(np.int64).max(axis=0), 1)

    schedule = [[] for _ in range(SETS)]
    goff_tab = np.zeros(SETS * 2 * N_REL, np.int64)
    off = 0
    for s in range(SETS):
        for p2 in range(2):
            for r in range(N_REL):
                key = (s * 2 + p2) * N_REL + r
                nt = int(ntiles[key])
                schedule[s].append((p2, r, nt, off))
                goff_tab[key] = off
                off += nt
    T_flat = off
    T_pad = ((T_flat + GB - 1) // GB) * GB

    starts = np.zeros(n_groups + 1, np.int64)
    starts[1:] = np.cumsum(np.bincount(gid_s, minlength=n_groups))
    per_core = []
    for c in range(NC):
        srcidx = np.zeros((T_pad, TILE_E), np.int64)
        scale = np.zeros((T_pad, TILE_E), np.float32)
        dcol = np.zeros((T_pad, TILE_E), np.int64)
        valid = np.zeros((T_pad, TILE_E), bool)
        for key in range(SETS * 2 * N_REL):
            gidx = c * (SETS * 2 * N_REL) + key
            sel = order[starts[gidx]:starts[gidx + 1]]
            k = len(sel)
            if not k:
                continue
            t0 = goff_tab[key]
            flat = np.arange(k)
            rr = flat // TILE_E + t0
            cc = flat % TILE_E
            srcidx[rr, cc] = src[sel]
            scale[rr, cc] = invc[dst[sel], et[sel]]
            dcol[rr, cc] = col[sel]
            valid[rr, cc] = True
        per_core.append((srcidx, scale, dcol, valid))
    return schedule, T_flat, T_pad, per_core, graph_of


def _make_xg(x, srcidx, scale):
    # p-major rows within each GB-batch: row = batch*GB*128 + p*GB + t
    T_pad = srcidx.shape[0]
    nb = T_pad // GB
    idx = srcidx.reshape(nb, GB, TILE_E).transpose(0, 2, 1).reshape(-1)
    sc = scale.reshape(nb, GB, TILE_E).transpose(0, 2, 1).reshape(-1)
    return np.ascontiguousarray(
        (x[idx] * sc[:, None]).astype(bf16))


def _make_oh(dcol, valid, dtype):
    T_pad = dcol.shape[0]
    nb = T_pad // GB
    dc = dcol.reshape(nb, GB, TILE_E).transpose(0, 2, 1).reshape(-1)
    va = valid.reshape(nb, GB, TILE_E).transpose(0, 2, 1).reshape(-1)
    oh = np.zeros((T_pad * TILE_E, GRAPH), np.float32)
    rows = np.where(va)[0]
    oh[rows, dc[rows]] = 1.0
    return np.ascontiguousarray(oh.astype(dtype))


def _make_weights(W_rel, W_root):
    tiles = []
    for r in range(N_REL):
        for k in range(3):
            tiles.append(W_rel[r, k * P:(k + 1) * P, :])
    for k in range(3):
        tiles.append(W_root[k * P:(k + 1) * P, :])
    return np.ascontiguousarray(np.stack(tiles).astype(bf16))


def _make_xt(x, graphs_c):
    # [SETS, 3, 128, 512] : xt[s, k, p, col] = x[node(col), k*128+p]
    out = np.zeros((SETS, 3, P, 2 * GRAPH), bf16)
    for s in range(SETS):
        nodes = np.concatenate([
            np.arange(graphs_c[2 * s] * GRAPH,
                      (graphs_c[2 * s] + 1) * GRAPH),
            np.arange(graphs_c[2 * s + 1] * GRAPH,
                      (graphs_c[2 * s + 1] + 1) * GRAPH)])
        xc = x[nodes]                      # [512, 384]
        out[s] = xc.T.reshape(3, P, 2 * GRAPH).astype(bf16)
    return np.ascontiguousarray(out)


def _build(schedule, T_pad, with_bias):
    nc = bacc.Bacc("TRN2", target_bir_lowering=False, debug=False,
                   enable_asserts=False, num_devices=NC)
    bfd = mybir.dt.bfloat16
    f32 = mybir.dt.float32

    xg_d = nc.dram_tensor("xg", [T_pad * P, D], bfd,
                          kind="ExternalInput").ap()
    oh_d = nc.dram_tensor("oh", [T_pad * P, GRAPH], OH_DTYPE_BIR,
                          kind="ExternalInput").ap()
    xt_d = nc.dram_tensor("xt", [SETS, 3, P, 2 * GRAPH], bfd,
                          kind="ExternalInput").ap()
    w_d = nc.dram_tensor("wstack", [30, P, H], bfd,
                         kind="ExternalInput").ap()
    bconv_d = nc.dram_tensor("bconv", [P, 3], bfd, kind="ExternalInput").ap()
    pool_out_d = nc.dram_tensor("pool_out", [P, 96], f32,
                                kind="ExternalOutput").ap()

    with tile.TileContext(nc) as tc, ExitStack() as ctx:
        const = ctx.enter_context(tc.tile_pool(name="const", bufs=1))
        ballpool = ctx.enter_context(tc.tile_pool(name="ballpool", bufs=2))
        gpool = ctx.enter_context(tc.tile_pool(name="gpool", bufs=8))
        ohpool = ctx.enter_context(tc.tile_pool(name="ohpool", bufs=8))
        hrpool = ctx.enter_context(tc.tile_pool(name="hrpool", bufs=4))
        utps = ctx.enter_context(tc.tile_pool(name="utps", bufs=2,
                                              space="PSUM"))
        hps = ctx.enter_context(tc.tile_pool(name="hps", bufs=2,
                                             space="PSUM"))

        pool_sb = const.tile([P, 96], f32, tag="pool")

        ev_engines = [nc.vector, nc.scalar]
        ev_i = 0

        gbufs = {}

        def get_batch(bi):
            if bi not in gbufs:
                G = gpool.tile([P, GB, D], bfd, tag="g")
                nc.sync.dma_start(
                    G[:],
                    xg_d[bi * GB * P:(bi + 1) * GB * P, :].rearrange(
                        "(p n) d -> p n d", p=P))
                O = ohpool.tile([P, GB, GRAPH], OH_DTYPE_BIR, tag="oh")
                nc.sync.dma_start(
                    O[:],
                    oh_d[bi * GB * P:(bi + 1) * GB * P, :].rearrange(
                        "(p n) d -> p n d", p=P))
                gbufs[bi] = (G, O)
                for k in [k for k in gbufs if k < bi - 6]:
                    del gbufs[k]
            return gbufs[bi]

        w_sb = const.tile([P, 30 * H], bfd, tag="w")
        nc.sync.dma_start(
            w_sb[:].rearrange("p (n d) -> p n d", n=30),
            w_d[:].rearrange("n p d -> p n d"))
        bconv_sb = const.tile([P, 3], bfd, tag="bconv")
        nc.sync.dma_start(bconv_sb[:], bconv_d[:])

        for s in range(SETS):
            ball = ballpool.tile([P, 30, 2 * GRAPH], bfd, tag="ball")
            nc.sync.dma_start(
                ball[:, 27:30, :],
                xt_d[s].rearrange("n p q -> p n q"))
            for (p2, r, nt, g0) in schedule[s]:
                uts = [utps.tile([P, GRAPH], f32, name=f"ut{k}",
                                 tag=f"ut{k}")
                       for k in range(3)]
                for i in range(nt):
                    gt = g0 + i
                    bi, o = divmod(gt, GB)
                    G, O = get_batch(bi)
                    for k in range(3):
                        nc.tensor.matmul(
                            out=uts[k][:],
                            lhsT=G[:, o, k * P:(k + 1) * P],
                            rhs=O[:, o, :],
                            start=(i == 0), stop=(i == nt - 1),
                        )
                for k in range(3):
                    ev = ev_engines[ev_i % len(ev_engines)]
                    ev_i += 1
                    dstap = ball[:, 3 * r + k,
                                 p2 * GRAPH:(p2 + 1) * GRAPH]
                    if ev is nc.scalar:
                        ev.copy(dstap, uts[k][:])
                    else:
                        ev.tensor_copy(dstap, uts[k][:])

            for hc in range(3):
                hT = hps.tile([P, 2 * GRAPH], f32, tag="ht")
                for slot in range(30):
                    nc.tensor.matmul(
                        out=hT[:],
                        lhsT=w_sb[:, slot * H + hc * P:
                                  slot * H + (hc + 1) * P],
                        rhs=ball[:, slot, :],
                        start=(slot == 0), stop=(slot == 29),
                    )
                hr = hrpool.tile([P, 2 * GRAPH], bfd, tag="hr")
                nc.scalar.activation(
                    out=hr[:], in_=hT[:],
                    func=mybir.ActivationFunctionType.Relu,
                    bias=bconv_sb[:, hc:hc + 1])
                nc.vector.tensor_reduce(
                    out=pool_sb[:, hc * 32 + 2 * s:hc * 32 + 2 * s + 2],
                    in_=hr[:].rearrange("p (g n) -> p g n", g=2),
                    op=mybir.AluOpType.add,
                    axis=mybir.AxisListType.X)

        nc.sync.dma_start(pool_out_d[:], pool_sb[:])

    nc.compile()
    return nc


def kernel(x, edge_index, edge_type, batch_size,
           W_rel, W_root, b_conv, W_lin1, b_lin1,
           bn_gamma, bn_beta, W_head, b_head,
           _trace_dir=None):
    x = np.asarray(x, np.float32)
    edge_index = np.asarray(edge_index)
    edge_type = np.asarray(edge_type)
    batch_size = int(batch_size)
    W_rel = np.asarray(W_rel, np.float32)
    W_root = np.asarray(W_root, np.float32)
    b_conv = np.asarray(b_conv, np.float32)

    src = edge_index[0].astype(np.int64)
    dst = edge_index[1].astype(np.int64)
    et = edge_type.astype(np.int64)

    schedule, T_flat, T_pad, per_core, graph_of = _preprocess(src, dst, et)
    nc = _build(schedule, T_pad, with_bias=bool(np.any(b_conv)))

    Wstack = _make_weights(W_rel, W_root)
    bconv = np.ascontiguousarray(b_conv.reshape(3, P).T.astype(bf16))

    in_maps = []
    for c in range(NC):
        srcidx, scale, dcol, valid = per_core[c]
        in_maps.append({
            "xg": _make_xg(x, srcidx, scale),
            "oh": _make_oh(dcol, valid, OH_DTYPE_NP),
            "xt": _make_xt(x, graph_of[c]),
            "wstack": Wstack,
            "bconv": bconv,
        })

    if _trace_dir is not None:
        res = run_bass_kernel_spmd(nc, in_maps, core_ids=list(range(NC)),
                                   trace=True, tmpdir=_trace_dir)
    else:
        res = run_bass_kernel_spmd(nc, in_maps, core_ids=list(range(NC)))
    global LAST_RES
    LAST_RES = res

    # host epilogue: mean-pool scale, lin1, BatchNorm (batch stats), head
    pooled = np.zeros((batch_size, H), np.float64)
    for c in range(NC):
        po = np.asarray(res.results[c]["pool_out"], np.float64)  # [128, 96]
        rows = np.asarray(graph_of[c], np.int64)
        for hc in range(3):
            pooled[rows, hc * P:(hc + 1) * P] = (
                po[:, hc * 32:(hc + 1) * 32].T / GRAPH)
    g = pooled @ np.asarray(W_lin1, np.float64) + np.asarray(b_lin1,
                                                            np.float64)
    mu = g.mean(axis=0)
    var = g.var(axis=0)
    g = (g - mu) / np.sqrt(var + BN_EPS) * np.asarray(bn_gamma, np.float64) \
        + np.asarray(bn_beta, np.float64)
    out = g @ np.asarray(W_head, np.float64) + np.asarray(b_head, np.float64)
    return np.squeeze(out.astype(np.float32))
